# revision 14
# baseline (speedup 1.0000x reference)
"""Trainium2 Bass kernel for nn_CLoss_inout: mean(1 - rowwise_dot(A, B)).

Full inputs A, B are [1048576, 128] f32. result = 1 - sum(A*B)/N (or
mean(A*B)+1 when flip). Data-parallel over 8 NeuronCores: core c gets rows
[c*131072, (c+1)*131072), viewed as [128 partitions x 131072 free] (order
of summation is irrelevant). Per tile of [128 x FT]: two HWDGE DMA loads,
one DVE tensor_mul (f32 inputs, bf16 product), and FT/512 PE matmuls
against a ones[128,1] stationary vector that accumulate per-column sums
into a single PSUM bank across the whole kernel. Tail: PSUM -> SBUF copy,
DMA the [1,512] partial out. The 8 per-core partials are summed on host.
"""

import numpy as np

N, D = 1048576, 128
M = 8                     # cores
ROWS = N // M             # 131072 rows per core
P = 128                   # SBUF partitions
FREE = ROWS * D // P      # 131072 f32 per partition per tensor
FT = 8192                 # tile free size: 128 x 8192 f32 = 4 MiB per DMA
BUFS = 2
MMF = 512                 # matmul moving free dim (one PSUM bank of f32)

TRACE = False             # test.py sets True to capture an NTFF profile
LAST = {}                 # stash of the most recent BassKernelResults

_cache = {}


def _ensure_path():
    import sys
    try:
        import concourse.bass  # noqa: F401
    except ImportError:
        sys.path.insert(0, "/opt/trn_rl_repo")


def build(free=FREE, ft=FT, bufs=BUFS):
    _ensure_path()
    import concourse.bacc as bacc
    import concourse.mybir as mybir
    from concourse.tile import TileContext

    assert free % ft == 0 and ft % MMF == 0
    nt = free // ft
    nj = ft // MMF
    # Bacc (not raw Bass): its compile pipeline splits multi-wait
    # instructions (TRN2 allows at most one sync wait per instruction).
    nc = bacc.Bacc(None, name="closs_inout")
    a = nc.dram_tensor("input_in", [P, free], mybir.dt.float32, kind="ExternalInput")
    b = nc.dram_tensor("input_out", [P, free], mybir.dt.float32, kind="ExternalInput")
    o = nc.dram_tensor("partial", [1, MMF], mybir.dt.float32, kind="ExternalOutput")

    with TileContext(nc) as tc:
        with (
            tc.tile_pool(name="pa", bufs=bufs) as pa,
            tc.tile_pool(name="pb", bufs=bufs) as pb,
            tc.tile_pool(name="pp", bufs=bufs) as pp,
            tc.tile_pool(name="misc", bufs=1) as misc,
            tc.tile_pool(name="psum", bufs=1, space="PSUM") as psum,
        ):
            ones = misc.tile([P, 1], mybir.dt.bfloat16)
            nc.gpsimd.memset(ones[:], 1.0)
            ps = psum.tile([1, MMF], mybir.dt.float32)
            for i in range(nt):
                at = pa.tile([P, ft], mybir.dt.float32)
                bt = pb.tile([P, ft], mybir.dt.float32)
                # Two physical HWDGE rings (SP + ACT): A-loads and B-loads
                # proceed in parallel instead of serializing on one FIFO.
                nc.sync.dma_start(out=at[:], in_=a[:, i * ft:(i + 1) * ft])
                nc.scalar.dma_start(out=bt[:], in_=b[:, i * ft:(i + 1) * ft])
                pt = pp.tile([P, ft], mybir.dt.bfloat16)
                nc.vector.tensor_mul(pt[:], at[:], bt[:])
                for j in range(nj):
                    # ps[0, n] += sum_p pt[p, j*MMF + n]
                    nc.tensor.matmul(
                        ps[:, :],
                        ones[:],
                        pt[:, j * MMF:(j + 1) * MMF],
                        start=(i == 0 and j == 0),
                        stop=(i == nt - 1 and j == nj - 1),
                    )
            out_sb = misc.tile([1, MMF], mybir.dt.float32)
            nc.vector.tensor_copy(out_sb[:], ps[:])
            nc.sync.dma_start(out=o[:], in_=out_sb[:])

    # Run the Bacc compile pipeline (wait splitting, reg alloc) before the
    # BIR is serialized for execution.
    nc.finalize()
    return nc


def _run_spmd(nc, in_maps, trace=False):
    """Execute `nc` SPMD on len(in_maps) cores with inputs pre-staged on
    device. Unlike bass_utils.run_bass_kernel_spmd (which feeds numpy into
    the jit call so each core starts executing as soon as its own H2D
    lands, while later cores' transfers still stream into HBM and steal
    bandwidth from the early cores), this device_puts every shard and
    blocks before launching the NEFF."""
    import jax
    import concourse.bass2jax as b2j
    import concourse.mybir as mybir
    from jax.experimental.shard_map import shard_map
    from jax.sharding import Mesh, NamedSharding, PartitionSpec

    b2j.install_neuronx_cc_hook()
    n = len(in_maps)
    partition_name = nc.partition_id_tensor.name if nc.partition_id_tensor else None

    in_names, out_names, out_avals = [], [], []
    for alloc in nc.m.functions[0].allocations:
        if not isinstance(alloc, mybir.MemoryLocationSet):
            continue
        name = alloc.memorylocations[0].name
        if alloc.kind == "ExternalInput":
            if name != partition_name:
                in_names.append(name)
        elif alloc.kind == "ExternalOutput":
            out_names.append(name)
            out_avals.append(
                jax.core.ShapedArray(
                    tuple(alloc.tensor_shape), mybir.dt.np(alloc.dtype)
                )
            )
    n_params = len(in_names)
    all_in = in_names + out_names + ([partition_name] if partition_name else [])

    def _body(*args):
        operands = list(args)
        if partition_name:
            operands.append(b2j.partition_id_tensor())
        return tuple(
            b2j._bass_exec_p.bind(
                *operands,
                out_avals=tuple(out_avals),
                in_names=tuple(all_in),
                out_names=tuple(out_names),
                lowering_input_output_aliases=(),
                sim_require_finite=True,
                sim_require_nnan=True,
                nc=nc,
            )
        )

    devices = jax.devices()[:n]
    mesh = Mesh(np.asarray(devices), ("core",))
    spec = PartitionSpec("core")
    n_outs = len(out_names)
    donate = tuple(range(n_params, n_params + n_outs))
    sharded = jax.jit(
        shard_map(
            _body,
            mesh=mesh,
            in_specs=(spec,) * (n_params + n_outs),
            out_specs=(spec,) * n_outs,
            check_rep=False,
        ),
        donate_argnums=donate,
        keep_unused=True,
    )

    sharding = NamedSharding(mesh, spec)
    concat_in = [
        np.concatenate([np.asarray(in_maps[c][nm]) for c in range(n)], axis=0)
        for nm in in_names
    ]

    def _zeros():
        zs = [
            jax.device_put(
                np.zeros((n * av.shape[0], *av.shape[1:]), av.dtype), sharding
            )
            for av in out_avals
        ]
        jax.block_until_ready(zs)
        return zs

    dev_in = [jax.device_put(x, sharding) for x in concat_in]
    jax.block_until_ready(dev_in)

    out_arrs = sharded(*dev_in, *_zeros())
    jax.block_until_ready(out_arrs)

    perf = None
    if trace:
        # Re-run under the NTFF hook: compile and H2D are out of the
        # window, so the capture sees only steady-state NEFF execution.
        perf = {}
        try:
            import tempfile

            from antenv.axon_hooks import get_axon_ntff_profile_hook

            hook = get_axon_ntff_profile_hook()
            if hook is not None:
                neff_dir = tempfile.mkdtemp()
                with hook(neff_dir, list(range(n))):
                    out_arrs = sharded(*dev_in, *_zeros())
                    jax.block_until_ready(out_arrs)
                perf["neff_dir"] = neff_dir
        except Exception as e:  # profiling must never break the run
            perf["error"] = repr(e)

    results = [
        {
            name: np.asarray(out_arrs[i]).reshape(n, *out_avals[i].shape)[c]
            for i, name in enumerate(out_names)
        }
        for c in range(n)
    ]
    return results, perf


def kernel(input_in, input_out, flip):
    _ensure_path()

    a = np.asarray(input_in, dtype=np.float32)
    b = np.asarray(input_out, dtype=np.float32)
    assert a.shape == (N, D) and b.shape == (N, D)

    nc = _cache.get("nc")
    if nc is None:
        nc = build()
        _cache["nc"] = nc

    in_maps = [
        {
            "input_in": np.ascontiguousarray(a[c * ROWS:(c + 1) * ROWS]).reshape(P, FREE),
            "input_out": np.ascontiguousarray(b[c * ROWS:(c + 1) * ROWS]).reshape(P, FREE),
        }
        for c in range(M)
    ]

    results, perf = _run_spmd(nc, in_maps, trace=TRACE)
    LAST["results"] = results
    LAST["perf"] = perf
    LAST["nc"] = nc

    total = float(np.sum([r["partial"].astype(np.float64).sum() for r in results]))
    mean_sim = total / float(N)
    if int(np.asarray(flip)) != 0:
        val = mean_sim + 1.0
    else:
        val = 1.0 - mean_sim
    return np.array(val, dtype=np.float32)


# revision 15
# speedup vs baseline: 1.1434x; 1.1434x over previous
"""Trainium2 Bass kernel for nn_CLoss_inout: mean(1 - rowwise_dot(A, B)).

Full inputs A, B are [1048576, 128] f32. result = 1 - sum(A*B)/N (or
mean(A*B)+1 when flip). Data-parallel over 8 NeuronCores: core c gets rows
[c*131072, (c+1)*131072), viewed as [128 partitions x 131072 free] (order
of summation is irrelevant). Per tile of [128 x FT]: two HWDGE DMA loads,
one DVE tensor_mul (f32 inputs, bf16 product), and FT/512 PE matmuls
against a ones[128,1] stationary vector that accumulate per-column sums
into a single PSUM bank across the whole kernel. Tail: PSUM -> SBUF copy,
DMA the [1,512] partial out. The 8 per-core partials are summed on host.
"""

import numpy as np

N, D = 1048576, 128
M = 8                     # cores
ROWS = N // M             # 131072 rows per core
P = 128                   # SBUF partitions
FREE = ROWS * D // P      # 131072 f32 per partition per tensor
FT = 4096                 # tile free size: 128 x 4096 f32 = 2 MiB per DMA
BUFS = 3
MMF = 512                 # matmul moving free dim (one PSUM bank of f32)

TRACE = False             # test.py sets True to capture an NTFF profile
LAST = {}                 # stash of the most recent BassKernelResults

_cache = {}


def _ensure_path():
    import sys
    try:
        import concourse.bass  # noqa: F401
    except ImportError:
        sys.path.insert(0, "/opt/trn_rl_repo")


def build(free=FREE, ft=FT, bufs=BUFS):
    _ensure_path()
    import concourse.bacc as bacc
    import concourse.mybir as mybir
    from concourse.tile import TileContext

    assert free % ft == 0 and ft % MMF == 0
    nt = free // ft
    nj = ft // MMF
    # Bacc (not raw Bass): its compile pipeline splits multi-wait
    # instructions (TRN2 allows at most one sync wait per instruction).
    nc = bacc.Bacc(None, name="closs_inout")
    a = nc.dram_tensor("input_in", [P, free], mybir.dt.float32, kind="ExternalInput")
    b = nc.dram_tensor("input_out", [P, free], mybir.dt.float32, kind="ExternalInput")
    o = nc.dram_tensor("partial", [1, MMF], mybir.dt.float32, kind="ExternalOutput")

    with TileContext(nc) as tc:
        with (
            tc.tile_pool(name="pa", bufs=bufs) as pa,
            tc.tile_pool(name="pb", bufs=bufs) as pb,
            tc.tile_pool(name="pp", bufs=bufs) as pp,
            tc.tile_pool(name="misc", bufs=1) as misc,
            tc.tile_pool(name="psum", bufs=1, space="PSUM") as psum,
        ):
            ones = misc.tile([P, 1], mybir.dt.bfloat16)
            nc.gpsimd.memset(ones[:], 1.0)
            ps = psum.tile([1, MMF], mybir.dt.float32)
            for i in range(nt):
                at = pa.tile([P, ft], mybir.dt.float32)
                bt = pb.tile([P, ft], mybir.dt.float32)
                # Two physical HWDGE rings (SP + ACT): A-loads and B-loads
                # proceed in parallel instead of serializing on one FIFO.
                nc.sync.dma_start(out=at[:], in_=a[:, i * ft:(i + 1) * ft])
                nc.scalar.dma_start(out=bt[:], in_=b[:, i * ft:(i + 1) * ft])
                pt = pp.tile([P, ft], mybir.dt.bfloat16)
                nc.vector.tensor_mul(pt[:], at[:], bt[:])
                for j in range(nj):
                    # ps[0, n] += sum_p pt[p, j*MMF + n]
                    nc.tensor.matmul(
                        ps[:, :],
                        ones[:],
                        pt[:, j * MMF:(j + 1) * MMF],
                        start=(i == 0 and j == 0),
                        stop=(i == nt - 1 and j == nj - 1),
                    )
            out_sb = misc.tile([1, MMF], mybir.dt.float32)
            nc.vector.tensor_copy(out_sb[:], ps[:])
            nc.sync.dma_start(out=o[:], in_=out_sb[:])

    # Run the Bacc compile pipeline (wait splitting, reg alloc) before the
    # BIR is serialized for execution.
    nc.finalize()
    return nc


def _run_spmd(nc, in_maps, trace=False):
    """Execute `nc` SPMD on len(in_maps) cores with inputs pre-staged on
    device. Unlike bass_utils.run_bass_kernel_spmd (which feeds numpy into
    the jit call so each core starts executing as soon as its own H2D
    lands, while later cores' transfers still stream into HBM and steal
    bandwidth from the early cores), this device_puts every shard and
    blocks before launching the NEFF."""
    import jax
    import concourse.bass2jax as b2j
    import concourse.mybir as mybir
    from jax.experimental.shard_map import shard_map
    from jax.sharding import Mesh, NamedSharding, PartitionSpec

    b2j.install_neuronx_cc_hook()
    n = len(in_maps)
    partition_name = nc.partition_id_tensor.name if nc.partition_id_tensor else None

    in_names, out_names, out_avals = [], [], []
    for alloc in nc.m.functions[0].allocations:
        if not isinstance(alloc, mybir.MemoryLocationSet):
            continue
        name = alloc.memorylocations[0].name
        if alloc.kind == "ExternalInput":
            if name != partition_name:
                in_names.append(name)
        elif alloc.kind == "ExternalOutput":
            out_names.append(name)
            out_avals.append(
                jax.core.ShapedArray(
                    tuple(alloc.tensor_shape), mybir.dt.np(alloc.dtype)
                )
            )
    n_params = len(in_names)
    all_in = in_names + out_names + ([partition_name] if partition_name else [])

    def _body(*args):
        operands = list(args)
        if partition_name:
            operands.append(b2j.partition_id_tensor())
        return tuple(
            b2j._bass_exec_p.bind(
                *operands,
                out_avals=tuple(out_avals),
                in_names=tuple(all_in),
                out_names=tuple(out_names),
                lowering_input_output_aliases=(),
                sim_require_finite=True,
                sim_require_nnan=True,
                nc=nc,
            )
        )

    devices = jax.devices()[:n]
    mesh = Mesh(np.asarray(devices), ("core",))
    spec = PartitionSpec("core")
    n_outs = len(out_names)
    donate = tuple(range(n_params, n_params + n_outs))
    sharded = jax.jit(
        shard_map(
            _body,
            mesh=mesh,
            in_specs=(spec,) * (n_params + n_outs),
            out_specs=(spec,) * n_outs,
            check_rep=False,
        ),
        donate_argnums=donate,
        keep_unused=True,
    )

    sharding = NamedSharding(mesh, spec)
    concat_in = [
        np.concatenate([np.asarray(in_maps[c][nm]) for c in range(n)], axis=0)
        for nm in in_names
    ]

    def _zeros():
        zs = [
            jax.device_put(
                np.zeros((n * av.shape[0], *av.shape[1:]), av.dtype), sharding
            )
            for av in out_avals
        ]
        jax.block_until_ready(zs)
        return zs

    dev_in = [jax.device_put(x, sharding) for x in concat_in]
    jax.block_until_ready(dev_in)

    out_arrs = sharded(*dev_in, *_zeros())
    jax.block_until_ready(out_arrs)

    perf = None
    if trace:
        # Re-run under the NTFF hook: compile and H2D are out of the
        # window, so the capture sees only steady-state NEFF execution.
        perf = {}
        try:
            import tempfile

            from antenv.axon_hooks import get_axon_ntff_profile_hook

            hook = get_axon_ntff_profile_hook()
            if hook is not None:
                neff_dir = tempfile.mkdtemp()
                with hook(neff_dir, list(range(n))):
                    out_arrs = sharded(*dev_in, *_zeros())
                    jax.block_until_ready(out_arrs)
                perf["neff_dir"] = neff_dir
        except Exception as e:  # profiling must never break the run
            perf["error"] = repr(e)

    results = [
        {
            name: np.asarray(out_arrs[i]).reshape(n, *out_avals[i].shape)[c]
            for i, name in enumerate(out_names)
        }
        for c in range(n)
    ]
    return results, perf


def kernel(input_in, input_out, flip):
    _ensure_path()

    a = np.asarray(input_in, dtype=np.float32)
    b = np.asarray(input_out, dtype=np.float32)
    assert a.shape == (N, D) and b.shape == (N, D)

    nc = _cache.get("nc")
    if nc is None:
        nc = build()
        _cache["nc"] = nc

    in_maps = [
        {
            "input_in": np.ascontiguousarray(a[c * ROWS:(c + 1) * ROWS]).reshape(P, FREE),
            "input_out": np.ascontiguousarray(b[c * ROWS:(c + 1) * ROWS]).reshape(P, FREE),
        }
        for c in range(M)
    ]

    results, perf = _run_spmd(nc, in_maps, trace=TRACE)
    LAST["results"] = results
    LAST["perf"] = perf
    LAST["nc"] = nc

    total = float(np.sum([r["partial"].astype(np.float64).sum() for r in results]))
    mean_sim = total / float(N)
    if int(np.asarray(flip)) != 0:
        val = mean_sim + 1.0
    else:
        val = 1.0 - mean_sim
    return np.array(val, dtype=np.float32)
